# revision 6
# baseline (speedup 1.0000x reference)
"""Bahdanau attention kernel for Trainium2, 8-core SPMD data-parallel over batch.

Problem shapes (hardcoded): values [32,2048,512] f32, query [32,512],
W1/W2 [512,512], b1/b2 [512], V [512,1], bV [1].

reference:
    proj_v = values @ W1 + b1            [B,T,U]
    proj_q = (query @ W2 + b2)[:,None,:] [B,1,U]
    score  = tanh(proj_v+proj_q) @ V + bV
    a      = softmax(score, axis=T)
    ctx    = sum_t a * values            [B,D]
    return ctx, a

Per-core plan (4 batches each):
  - load values[b] natural [t,d] -> SBUF (fast contiguous DMA)
  - PE-transpose 128x128 tiles -> vT [d,t] (PSUM->SBUF via DVE copies)
  - mm1: pv^T[u,t] = sum_d W1[d,u] * vT[d,t]  (fp32r, N=512, full PE rate)
  - ACT: s[u,t] = tanh(pv^T + beta[u,b]) where beta = (q@W2)+b1+b2 per batch
  - V-dot: score[1,t] = sum_u V[u]*s[u,t] (M=1 matmuls, accumulate over u-chunks)
  - ACT: e = exp(score) row-wise, with fused accumulated Z partials
  - PE-transpose e row -> e column layout [128,16]
  - context: sum_tc ecol[:,tc].T @ vnat[:,tc,:] -> [1,512]; scale by 1/Z
  - bV is skipped: softmax is shift-invariant, so it cancels exactly.
"""

import sys

if "/opt/trn_rl_repo" not in sys.path:
    sys.path.insert(0, "/opt/trn_rl_repo")

from contextlib import ExitStack

import numpy as np

import concourse.bacc as bacc
import concourse.tile as tile
from concourse import mybir
from concourse.bass_utils import run_bass_kernel_spmd
from concourse.masks import make_identity

F32 = mybir.dt.float32
MMDT = mybir.dt.float32r  # streaming dtype for matmuls (full PE rate at N>=256)
AF = mybir.ActivationFunctionType

B, T, D, U = 32, 2048, 512, 512
NCORES = 8
BL = B // NCORES  # batches per core
P = 128
DCH = D // P  # 4 d-chunks
UCH = U // P  # 4 u-chunks
TCH = T // P  # 16 t-chunks
TGS = 512     # matmul free-dim block over t
TG = T // TGS  # 4 t-blocks

_CACHE = {}


def _mm(ap):
    return ap.bitcast(MMDT)


def build(debug=False):
    nc = bacc.Bacc("TRN2", target_bir_lowering=False, debug=debug)

    vals = nc.dram_tensor("values_l", [BL, T, D], F32, kind="ExternalInput").ap()
    qry = nc.dram_tensor("query_l", [BL, D], F32, kind="ExternalInput").ap()
    w1 = nc.dram_tensor("W1", [D, U], F32, kind="ExternalInput").ap()
    w2 = nc.dram_tensor("W2", [D, U], F32, kind="ExternalInput").ap()
    b1 = nc.dram_tensor("b1", [U], F32, kind="ExternalInput").ap()
    b2 = nc.dram_tensor("b2", [U], F32, kind="ExternalInput").ap()
    vv = nc.dram_tensor("V", [U, 1], F32, kind="ExternalInput").ap()
    ctx_out = nc.dram_tensor("ctx_out", [BL, D], F32, kind="ExternalOutput").ap()
    attn_out = nc.dram_tensor("attn_out", [BL, T], F32, kind="ExternalOutput").ap()

    with ExitStack() as ctx:
        tc = ctx.enter_context(tile.TileContext(nc))

        consts = ctx.enter_context(tc.tile_pool(name="consts", bufs=1))
        vnat_p = ctx.enter_context(tc.tile_pool(name="vnat", bufs=2))

        ident = consts.tile([P, P], F32)
        make_identity(nc, ident)
        identr = consts.tile([P, P], MMDT)
        nc.vector.tensor_copy(identr, ident)

        # batch-0 values go first: the SP engine issues DMAs serially
        # (~1us each) and the first transposes need this data ASAP.
        vnat0 = vnat_p.tile([P, TCH, D], MMDT, tag="vnat", name="vnat_first")
        vnat0_src = vals[0].rearrange("(t p) d -> p t d", p=P).bitcast(MMDT)
        for tg in range(TG):
            nc.sync.dma_start(
                out=vnat0[:, tg * 4 : (tg + 1) * 4, :],
                in_=vnat0_src[:, tg * 4 : (tg + 1) * 4, :],
            )

        W1sb = consts.tile([P, DCH, U], F32)
        nc.sync.dma_start(out=W1sb, in_=w1.rearrange("(dc p) u -> p dc u", p=P))
        W2sb = consts.tile([P, DCH, U], F32)
        nc.sync.dma_start(out=W2sb, in_=w2.rearrange("(dc p) u -> p dc u", p=P))

        b1row = consts.tile([1, U], F32)
        nc.sync.dma_start(out=b1row, in_=b1.rearrange("(one u) -> one u", one=1))
        b2row = consts.tile([1, U], F32)
        nc.sync.dma_start(out=b2row, in_=b2.rearrange("(one u) -> one u", one=1))
        b12row = consts.tile([1, U], F32)
        nc.vector.tensor_add(b12row, b1row, b2row)
        vrow = consts.tile([1, U], F32)
        nc.sync.dma_start(out=vrow, in_=vv.rearrange("u one -> one u"))
        qsb = consts.tile([BL, D], F32)
        nc.sync.dma_start(out=qsb, in_=qry)

        # W1 rounded to f32r once so fp32r matmuls accept it as input
        W1r = consts.tile([P, DCH, U], MMDT)
        nc.vector.tensor_copy(W1r, W1sb)

        Vsb = consts.tile([P, UCH], MMDT)
        b12c = consts.tile([P, UCH], F32)
        qTc = consts.tile([P, DCH, BL], F32)
        beta = consts.tile([P, UCH, BL], F32)

        with tc.tile_pool(name="setup_ps", bufs=1, space="PSUM") as sps:
            ps_v = sps.tile([P, UCH], F32, tag="ps_v")
            for i in range(UCH):
                nc.tensor.matmul(
                    ps_v[:, i : i + 1], vrow[0:1, i * P : (i + 1) * P],
                    ident[0:1, 0:1], is_transpose=True,
                    start=(i == 0), stop=(i == UCH - 1),
                )
            nc.vector.tensor_copy(Vsb, ps_v)

            ps_b = sps.tile([P, UCH], F32, tag="ps_b")
            for i in range(UCH):
                nc.tensor.matmul(
                    ps_b[:, i : i + 1], b12row[0:1, i * P : (i + 1) * P],
                    ident[0:1, 0:1], is_transpose=True,
                    start=(i == 0), stop=(i == UCH - 1),
                )
            nc.vector.tensor_copy(b12c, ps_b)

            ps_q = sps.tile([P, DCH, BL], F32, tag="ps_q")
            for dc in range(DCH):
                nc.tensor.matmul(
                    ps_q[:, dc, :], qsb[0:BL, dc * P : (dc + 1) * P],
                    ident[0:BL, 0:BL], is_transpose=True,
                    start=(dc == 0), stop=(dc == DCH - 1),
                )
            nc.vector.tensor_copy(qTc, ps_q)

            # beta[u, b] = sum_d W2[d,u] q[b,d] + b1[u] + b2[u]
            for uc in range(UCH):
                pq = sps.tile([P, BL], F32, tag="pq")
                for dc in range(DCH):
                    nc.tensor.matmul(
                        pq, W2sb[:, dc, uc * P : (uc + 1) * P], qTc[:, dc, :],
                        start=(dc == 0), stop=(dc == DCH - 1),
                    )
                nc.vector.tensor_scalar_add(beta[:, uc, :], pq, b12c[:, uc : uc + 1])

        vT_p = ctx.enter_context(tc.tile_pool(name="vT", bufs=1))
        s_p = ctx.enter_context(tc.tile_pool(name="s", bufs=1))
        small = ctx.enter_context(tc.tile_pool(name="small", bufs=2))
        ptr = ctx.enter_context(tc.tile_pool(name="ptr", bufs=2, space="PSUM"))
        ppv = ctx.enter_context(tc.tile_pool(name="ppv", bufs=3, space="PSUM"))
        psc = ctx.enter_context(tc.tile_pool(name="psc", bufs=2, space="PSUM"))
        pec = ctx.enter_context(tc.tile_pool(name="pec", bufs=1, space="PSUM"))

        def load_vnat(b):
            vnat = vnat_p.tile([P, TCH, D], MMDT, tag="vnat", name=f"vnat{b}")
            src_ap = vals[b].rearrange("(t p) d -> p t d", p=P).bitcast(MMDT)
            for tg in range(TG):
                nc.sync.dma_start(
                    out=vnat[:, tg * 4 : (tg + 1) * 4, :],
                    in_=src_ap[:, tg * 4 : (tg + 1) * 4, :],
                )
            return vnat

        def transpose_vals(b, vnat):
            # transpose values -> vT [d-part, dc, t] (f32r transpose: 1.5cyc/row)
            vT = vT_p.tile([P, DCH, T], MMDT, tag="vT", name=f"vT{b}")
            for dc in range(DCH):
                for tg in range(TG):
                    tr = ptr.tile([P, TGS], MMDT, tag="tr", name=f"tr{b}_{dc}_{tg}")
                    for t4 in range(4):
                        ti = tg * 4 + t4
                        nc.tensor.matmul(
                            tr[:, t4 * P : (t4 + 1) * P],
                            vnat[:, ti, dc * P : (dc + 1) * P],
                            identr, is_transpose=True,
                            start=(t4 == 0), stop=(t4 == 3),
                        )
                    nc.vector.tensor_copy(vT[:, dc, tg * TGS : (tg + 1) * TGS], tr)
            return vT

        for b in range(BL):
            if b == 0:
                vnat_cur = vnat0
                vT_cur = transpose_vals(0, vnat_cur)
            vnat, vT = vnat_cur, vT_cur
            if b + 1 < BL:
                vnat_nxt = load_vnat(b + 1)

            # mm1 + tanh: s[u-part, uc, t] = tanh(pv^T + beta)
            s = s_p.tile([P, UCH, T], MMDT, tag="s")
            for tg in range(TG):
                for uc in range(UCH):
                    pv = ppv.tile([P, TGS], F32, tag="pv")
                    for dc in range(DCH):
                        nc.tensor.matmul(
                            pv, W1r[:, dc, uc * P : (uc + 1) * P],
                            vT[:, dc, tg * TGS : (tg + 1) * TGS],
                            start=(dc == 0), stop=(dc == DCH - 1),
                        )
                    nc.scalar.activation(
                        s[:, uc, tg * TGS : (tg + 1) * TGS], pv, AF.Tanh,
                        bias=beta[:, uc, b : b + 1],
                    )

            # next batch's transposes fill the PE while softmax chain runs
            if b + 1 < BL:
                vT_cur = transpose_vals(b + 1, vnat_nxt)
                vnat_cur = vnat_nxt

            # V-dot + exp (+ partial sums of Z)
            e_row = small.tile([1, T], F32, tag="e_row")
            zpart = small.tile([1, TG], F32, tag="zpart")
            for tg in range(TG):
                sc = psc.tile([1, TGS], F32, tag="sc")
                for uc in range(UCH):
                    nc.tensor.matmul(
                        sc, Vsb[:, uc : uc + 1],
                        s[:, uc, tg * TGS : (tg + 1) * TGS],
                        start=(uc == 0), stop=(uc == UCH - 1),
                    )
                nc.scalar.activation(
                    e_row[:, tg * TGS : (tg + 1) * TGS], sc, AF.Exp,
                    accum_out=zpart[:, tg : tg + 1],
                )

            z = small.tile([1, 1], F32, tag="z")
            nc.vector.tensor_reduce(
                z, zpart, axis=mybir.AxisListType.X, op=mybir.AluOpType.add
            )
            rz = small.tile([1, 1], F32, tag="rz")
            nc.vector.reciprocal(rz, z)

            # e row -> column layout [p, tc]
            pecol = pec.tile([P, TCH], F32, tag="pecol")
            for ti in range(TCH):
                nc.tensor.matmul(
                    pecol[:, ti : ti + 1], e_row[0:1, ti * P : (ti + 1) * P],
                    ident[0:1, 0:1], is_transpose=True,
                    start=(ti == 0), stop=(ti == TCH - 1),
                )
            ecol = small.tile([P, TCH], MMDT, tag="ecol")
            nc.vector.tensor_copy(ecol, pecol)

            # context: sum_t e[t] * values[t, :]
            cx = psc.tile([1, D], F32, tag="sc", name=f"cx{b}")
            for ti in range(TCH):
                nc.tensor.matmul(
                    cx, ecol[:, ti : ti + 1], vnat[:, ti, :],
                    start=(ti == 0), stop=(ti == TCH - 1),
                )
            ctx_sb = small.tile([1, D], F32, tag="ctx_sb")
            nc.scalar.activation(ctx_sb, cx, AF.Copy, scale=rz[0:1, 0:1])
            nc.sync.dma_start(out=ctx_out[b : b + 1, :], in_=ctx_sb)

            # normalize attention weights in place and store
            nc.vector.tensor_scalar_mul(e_row, e_row, rz[0:1, 0:1])
            nc.sync.dma_start(out=attn_out[b : b + 1, :], in_=e_row)

    nc.compile()
    return nc


def _get_nc():
    if "nc" not in _CACHE:
        _CACHE["nc"] = build()
    return _CACHE["nc"]


def kernel(values, query, W1, b1, W2, b2, V, bV):
    nc = _get_nc()
    values = np.asarray(values, dtype=np.float32)
    query = np.asarray(query, dtype=np.float32)
    shared = {
        "W1": np.ascontiguousarray(W1, dtype=np.float32),
        "W2": np.ascontiguousarray(W2, dtype=np.float32),
        "b1": np.ascontiguousarray(b1, dtype=np.float32),
        "b2": np.ascontiguousarray(b2, dtype=np.float32),
        "V": np.ascontiguousarray(V, dtype=np.float32),
    }
    in_maps = []
    for c in range(NCORES):
        sl = slice(c * BL, (c + 1) * BL)
        in_maps.append(
            {
                "values_l": np.ascontiguousarray(values[sl]),
                "query_l": np.ascontiguousarray(query[sl]),
                **shared,
            }
        )
    res = run_bass_kernel_spmd(nc, in_maps, core_ids=list(range(NCORES)))
    context = np.concatenate([res.results[c]["ctx_out"] for c in range(NCORES)], 0)
    attn = np.concatenate([res.results[c]["attn_out"] for c in range(NCORES)], 0)
    return context, attn.reshape(B, T, 1)


# revision 7
# speedup vs baseline: 1.0902x; 1.0902x over previous
"""Bahdanau attention kernel for Trainium2, 8-core SPMD data-parallel over batch.

Problem shapes (hardcoded): values [32,2048,512] f32, query [32,512],
W1/W2 [512,512], b1/b2 [512], V [512,1], bV [1].

reference:
    proj_v = values @ W1 + b1            [B,T,U]
    proj_q = (query @ W2 + b2)[:,None,:] [B,1,U]
    score  = tanh(proj_v+proj_q) @ V + bV
    a      = softmax(score, axis=T)
    ctx    = sum_t a * values            [B,D]
    return ctx, a

Per-core plan (4 batches each):
  - load values[b] natural [t,d] -> SBUF (fast contiguous DMA)
  - PE-transpose 128x128 tiles -> vT [d,t] (PSUM->SBUF via DVE copies)
  - mm1: pv^T[u,t] = sum_d W1[d,u] * vT[d,t]  (fp32r, N=512, full PE rate)
  - ACT: s[u,t] = tanh(pv^T + beta[u,b]) where beta = (q@W2)+b1+b2 per batch
  - V-dot: score[1,t] = sum_u V[u]*s[u,t] (M=1 matmuls, accumulate over u-chunks)
  - ACT: e = exp(score) row-wise, with fused accumulated Z partials
  - PE-transpose e row -> e column layout [128,16]
  - context: sum_tc ecol[:,tc].T @ vnat[:,tc,:] -> [1,512]; scale by 1/Z
  - bV is skipped: softmax is shift-invariant, so it cancels exactly.
"""

import sys

if "/opt/trn_rl_repo" not in sys.path:
    sys.path.insert(0, "/opt/trn_rl_repo")

from contextlib import ExitStack

import numpy as np

import concourse.bacc as bacc
import concourse.tile as tile
from concourse import mybir
from concourse.bass_utils import run_bass_kernel_spmd
from concourse.masks import make_identity

F32 = mybir.dt.float32
MMDT = mybir.dt.float32r  # streaming dtype for matmuls (full PE rate at N>=256)
AF = mybir.ActivationFunctionType

B, T, D, U = 32, 2048, 512, 512
NCORES = 8
BL = B // NCORES  # batches per core
P = 128
DCH = D // P  # 4 d-chunks
UCH = U // P  # 4 u-chunks
TCH = T // P  # 16 t-chunks
TGS = 512     # matmul free-dim block over t
TG = T // TGS  # 4 t-blocks

_CACHE = {}


def _mm(ap):
    return ap.bitcast(MMDT)


def build(debug=False):
    nc = bacc.Bacc("TRN2", target_bir_lowering=False, debug=debug)

    vals = nc.dram_tensor("values_l", [BL, T, D], F32, kind="ExternalInput").ap()
    qry = nc.dram_tensor("query_l", [BL, D], F32, kind="ExternalInput").ap()
    w1 = nc.dram_tensor("W1", [D, U], F32, kind="ExternalInput").ap()
    w2 = nc.dram_tensor("W2", [D, U], F32, kind="ExternalInput").ap()
    b1 = nc.dram_tensor("b1", [U], F32, kind="ExternalInput").ap()
    b2 = nc.dram_tensor("b2", [U], F32, kind="ExternalInput").ap()
    vv = nc.dram_tensor("V", [U, 1], F32, kind="ExternalInput").ap()
    ctx_out = nc.dram_tensor("ctx_out", [BL, D], F32, kind="ExternalOutput").ap()
    attn_out = nc.dram_tensor("attn_out", [BL, T], F32, kind="ExternalOutput").ap()

    with ExitStack() as ctx:
        tc = ctx.enter_context(tile.TileContext(nc))

        consts = ctx.enter_context(tc.tile_pool(name="consts", bufs=1))
        vnat_p = ctx.enter_context(tc.tile_pool(name="vnat", bufs=2))
        vT_p = ctx.enter_context(tc.tile_pool(name="vT", bufs=1))
        s_p = ctx.enter_context(tc.tile_pool(name="s", bufs=1))
        small = ctx.enter_context(tc.tile_pool(name="small", bufs=2))
        ptr = ctx.enter_context(tc.tile_pool(name="ptr", bufs=2, space="PSUM"))
        ppv = ctx.enter_context(tc.tile_pool(name="ppv", bufs=3, space="PSUM"))
        psc = ctx.enter_context(tc.tile_pool(name="psc", bufs=2, space="PSUM"))
        pec = ctx.enter_context(tc.tile_pool(name="pec", bufs=1, space="PSUM"))

        ident = consts.tile([P, P], F32)
        make_identity(nc, ident)
        identr = consts.tile([P, P], MMDT)
        nc.vector.tensor_copy(identr, ident)

        def load_vnat(b, name):
            # chunked so the first transposes can start before the whole 4MB lands
            vnat = vnat_p.tile([P, TCH, D], MMDT, tag="vnat", name=name)
            src_ap = vals[b].rearrange("(t p) d -> p t d", p=P).bitcast(MMDT)
            for tg in range(TG):
                nc.sync.dma_start(
                    out=vnat[:, tg * 4 : (tg + 1) * 4, :],
                    in_=src_ap[:, tg * 4 : (tg + 1) * 4, :],
                )
            return vnat

        # batch-0 values go first: SP issues DMAs serially (~0.7us each) and
        # the first transposes need this data ASAP
        vnat0 = load_vnat(0, "vnat_first")

        W1sb = consts.tile([P, DCH, U], F32)
        nc.sync.dma_start(out=W1sb, in_=w1.rearrange("(dc p) u -> p dc u", p=P))
        # W1 rounded to f32r once so fp32r matmuls accept it as input
        W1r = consts.tile([P, DCH, U], MMDT)
        nc.vector.tensor_copy(W1r, W1sb)

        qsb = consts.tile([BL, D], F32)
        nc.sync.dma_start(out=qsb, in_=qry)
        b1row = consts.tile([1, U], F32)
        nc.sync.dma_start(out=b1row, in_=b1.rearrange("(one u) -> one u", one=1))
        b2row = consts.tile([1, U], F32)
        nc.sync.dma_start(out=b2row, in_=b2.rearrange("(one u) -> one u", one=1))
        b12row = consts.tile([1, U], F32)
        nc.vector.tensor_add(b12row, b1row, b2row)
        vrow = consts.tile([1, U], F32)
        nc.sync.dma_start(out=vrow, in_=vv.rearrange("u one -> one u"))
        W2sb = consts.tile([P, DCH, U], F32)
        nc.sync.dma_start(out=W2sb, in_=w2.rearrange("(dc p) u -> p dc u", p=P))

        Vsb = consts.tile([P, UCH], MMDT)
        b12c = consts.tile([P, UCH], F32)
        qTc = consts.tile([P, DCH, BL], F32)
        beta = consts.tile([P, UCH, BL], F32)

        def transpose_vals(b, vnat):
            # transpose values -> vT [d-part, dc, t] (f32r transpose: 1.5cyc/row)
            # tg-outer so batch 0 chases its chunked DMA
            vT = vT_p.tile([P, DCH, T], MMDT, tag="vT", name=f"vT{b}")
            for tg in range(TG):
                for dc in range(DCH):
                    tr = ptr.tile([P, TGS], MMDT, tag="tr", name=f"tr{b}_{dc}_{tg}")
                    for t4 in range(4):
                        ti = tg * 4 + t4
                        nc.tensor.matmul(
                            tr[:, t4 * P : (t4 + 1) * P],
                            vnat[:, ti, dc * P : (dc + 1) * P],
                            identr, is_transpose=True,
                            start=(t4 == 0), stop=(t4 == 3),
                        )
                    nc.vector.tensor_copy(vT[:, dc, tg * TGS : (tg + 1) * TGS], tr)
            return vT

        def setup_small():
            # V, b1+b2, q transposed to column layouts; beta = (q@W2)^T + b1 + b2.
            # PSUM comes from the pec/ppv pools (no separate setup pool: a
            # released setup pool would chain the first batch's transposes
            # behind the whole setup via bank reuse).
            ps_v = pec.tile([P, TCH], F32, tag="pecol", name="ps_v")
            for i in range(UCH):
                nc.tensor.matmul(
                    ps_v[:, i : i + 1], vrow[0:1, i * P : (i + 1) * P],
                    ident[0:1, 0:1], is_transpose=True,
                    start=(i == 0), stop=(i == UCH - 1),
                )
            nc.vector.tensor_copy(Vsb, ps_v[:, 0:UCH])

            ps_b = pec.tile([P, TCH], F32, tag="pecol", name="ps_b")
            for i in range(UCH):
                nc.tensor.matmul(
                    ps_b[:, i : i + 1], b12row[0:1, i * P : (i + 1) * P],
                    ident[0:1, 0:1], is_transpose=True,
                    start=(i == 0), stop=(i == UCH - 1),
                )
            nc.vector.tensor_copy(b12c, ps_b[:, 0:UCH])

            ps_q = pec.tile([P, TCH], F32, tag="pecol", name="ps_q")
            for dc in range(DCH):
                nc.tensor.matmul(
                    ps_q[:, dc * BL : (dc + 1) * BL], qsb[0:BL, dc * P : (dc + 1) * P],
                    ident[0:BL, 0:BL], is_transpose=True,
                    start=(dc == 0), stop=(dc == DCH - 1),
                )
            nc.vector.tensor_copy(qTc, ps_q)

            for uc in range(UCH):
                pq = ppv.tile([P, BL], F32, tag="pv", name=f"pq{uc}")
                for dc in range(DCH):
                    nc.tensor.matmul(
                        pq, W2sb[:, dc, uc * P : (uc + 1) * P], qTc[:, dc, :],
                        start=(dc == 0), stop=(dc == DCH - 1),
                    )
                nc.vector.tensor_scalar_add(beta[:, uc, :], pq, b12c[:, uc : uc + 1])

        for b in range(BL):
            if b == 0:
                vnat_cur = vnat0
                vT_cur = transpose_vals(0, vnat_cur)
                setup_small()
            vnat, vT = vnat_cur, vT_cur
            if b + 1 < BL:
                vnat_nxt = load_vnat(b + 1, f"vnat{b+1}")

            # mm1 + tanh: s[u-part, uc, t] = tanh(pv^T + beta)
            s = s_p.tile([P, UCH, T], MMDT, tag="s")
            for tg in range(TG):
                for uc in range(UCH):
                    pv = ppv.tile([P, TGS], F32, tag="pv")
                    for dc in range(DCH):
                        nc.tensor.matmul(
                            pv, W1r[:, dc, uc * P : (uc + 1) * P],
                            vT[:, dc, tg * TGS : (tg + 1) * TGS],
                            start=(dc == 0), stop=(dc == DCH - 1),
                        )
                    nc.scalar.activation(
                        s[:, uc, tg * TGS : (tg + 1) * TGS], pv, AF.Tanh,
                        bias=beta[:, uc, b : b + 1],
                    )

            # next batch's transposes fill the PE while softmax chain runs
            if b + 1 < BL:
                vT_cur = transpose_vals(b + 1, vnat_nxt)
                vnat_cur = vnat_nxt

            # V-dot + exp (+ partial sums of Z)
            e_row = small.tile([1, T], F32, tag="e_row")
            zpart = small.tile([1, TG], F32, tag="zpart")
            for tg in range(TG):
                sc = psc.tile([1, TGS], F32, tag="sc")
                for uc in range(UCH):
                    nc.tensor.matmul(
                        sc, Vsb[:, uc : uc + 1],
                        s[:, uc, tg * TGS : (tg + 1) * TGS],
                        start=(uc == 0), stop=(uc == UCH - 1),
                    )
                nc.scalar.activation(
                    e_row[:, tg * TGS : (tg + 1) * TGS], sc, AF.Exp,
                    accum_out=zpart[:, tg : tg + 1],
                )

            z = small.tile([1, 1], F32, tag="z")
            nc.vector.tensor_reduce(
                z, zpart, axis=mybir.AxisListType.X, op=mybir.AluOpType.add
            )
            rz = small.tile([1, 1], F32, tag="rz")
            nc.vector.reciprocal(rz, z)

            # e row -> column layout [p, tc]
            pecol = pec.tile([P, TCH], F32, tag="pecol")
            for ti in range(TCH):
                nc.tensor.matmul(
                    pecol[:, ti : ti + 1], e_row[0:1, ti * P : (ti + 1) * P],
                    ident[0:1, 0:1], is_transpose=True,
                    start=(ti == 0), stop=(ti == TCH - 1),
                )
            ecol = small.tile([P, TCH], MMDT, tag="ecol")
            nc.vector.tensor_copy(ecol, pecol)

            # context: sum_t e[t] * values[t, :]
            cx = psc.tile([1, D], F32, tag="sc", name=f"cx{b}")
            for ti in range(TCH):
                nc.tensor.matmul(
                    cx, ecol[:, ti : ti + 1], vnat[:, ti, :],
                    start=(ti == 0), stop=(ti == TCH - 1),
                )
            ctx_sb = small.tile([1, D], F32, tag="ctx_sb")
            nc.scalar.activation(ctx_sb, cx, AF.Copy, scale=rz[0:1, 0:1])
            nc.sync.dma_start(out=ctx_out[b : b + 1, :], in_=ctx_sb)

            # normalize attention weights in place and store
            nc.vector.tensor_scalar_mul(e_row, e_row, rz[0:1, 0:1])
            nc.sync.dma_start(out=attn_out[b : b + 1, :], in_=e_row)

    nc.compile()
    return nc


def _get_nc():
    if "nc" not in _CACHE:
        _CACHE["nc"] = build()
    return _CACHE["nc"]


def kernel(values, query, W1, b1, W2, b2, V, bV):
    nc = _get_nc()
    values = np.asarray(values, dtype=np.float32)
    query = np.asarray(query, dtype=np.float32)
    shared = {
        "W1": np.ascontiguousarray(W1, dtype=np.float32),
        "W2": np.ascontiguousarray(W2, dtype=np.float32),
        "b1": np.ascontiguousarray(b1, dtype=np.float32),
        "b2": np.ascontiguousarray(b2, dtype=np.float32),
        "V": np.ascontiguousarray(V, dtype=np.float32),
    }
    in_maps = []
    for c in range(NCORES):
        sl = slice(c * BL, (c + 1) * BL)
        in_maps.append(
            {
                "values_l": np.ascontiguousarray(values[sl]),
                "query_l": np.ascontiguousarray(query[sl]),
                **shared,
            }
        )
    res = run_bass_kernel_spmd(nc, in_maps, core_ids=list(range(NCORES)))
    context = np.concatenate([res.results[c]["ctx_out"] for c in range(NCORES)], 0)
    attn = np.concatenate([res.results[c]["attn_out"] for c in range(NCORES)], 0)
    return context, attn.reshape(B, T, 1)
